# revision 56
# baseline (speedup 1.0000x reference)
"""Trainium2 Bass kernel for a sparse-attention (sliding-window) transformer block.

Reference computation (per batch b, token t):
    x = x + attn(rmsnorm(x, ln1_w));  attn = (windowed_softmax(qk)·v * sigmoid(gate)) @ out_w.T
    out = x + swiglu_ffn(rmsnorm(x, ln2_w))

Sharding: token-parallel across 8 cores (B=2 x 4 chunks of 512 tokens).  Each
core receives its 512 tokens plus the previous 256 tokens (sliding-window halo,
zeros for the first chunk) and recomputes K/V on the halo -> no collectives.

v2 versus the baseline:
  - all weights + most activations in bf16 (half the HBM traffic, 2x/4x DVE
    modes); the residual stream and PSUM accumulation stay fp32.
  - weights are pre-tiled on the host into the exact SBUF layouts so every
    DMA is partition-major contiguous (>=1KB descriptors).
  - DMA spread across all three issueable queues (SP / Act HWDGE + Pool
    SWDGE) round-robin, with FFN weights prefetched during attention.
  - softmax denominators are batched per query-half (one reciprocal + a
    one-hot PE matmul broadcast) instead of per-head gpsimd broadcasts.
  - elementwise work balanced across Act/DVE so the PE stream never starves.
"""

import os
import sys

import numpy as np

if "/opt/trn_rl_repo" not in sys.path:
    sys.path.insert(0, "/opt/trn_rl_repo")

# ---- problem constants (hardcoded; kernel.py must be self-contained) ----
D = 1024          # d_model
NH = 16           # heads
DH = 64           # head dim
DFF = 4096        # ffn hidden
WIN = 256         # sliding window
B, T = 2, 2048
EPS = 1e-6

NCORES = 8
CHUNK = 512       # own tokens per core
HALO = 256        # preceding-token halo
LT = CHUNK + HALO  # 768 local tokens (halo first)
P = 128
ND = D // P       # 8
NF = DFF // P     # 32

EXP_SHIFT = -20.0  # constant shift inside exp; softmax-invariant
SCALE = DH ** -0.5

_CACHE = {}


# --------------------------------------------------------------------------
# program builder
# --------------------------------------------------------------------------

def build_program():
    import concourse.bacc as bacc
    import concourse.tile as tile
    from concourse import mybir

    f32 = mybir.dt.float32
    bf = mybir.dt.bfloat16

    nc = bacc.Bacc("TRN2", target_bir_lowering=False, debug=False,
                   num_devices=NCORES)

    io = {}
    # activations
    io["xb"] = nc.dram_tensor("xb", [P, ND, LT], bf, kind="ExternalInput").ap()
    io["xf"] = nc.dram_tensor("xf", [P, ND, CHUNK], f32,
                              kind="ExternalInput").ap()
    # attention weights, pre-tiled [partition, tiles..., free]
    io["wq"] = nc.dram_tensor("wq", [P, ND, ND, P], bf, kind="ExternalInput").ap()
    io["wk"] = nc.dram_tensor("wk", [P, ND, ND, P], bf, kind="ExternalInput").ap()
    io["wv"] = nc.dram_tensor("wv", [P, 4, ND, 256], bf, kind="ExternalInput").ap()
    io["wgt"] = nc.dram_tensor("wgt", [P, ND, ND, P], bf, kind="ExternalInput").ap()
    io["wo_a"] = nc.dram_tensor("wo_a", [P, ND, ND, P], bf, kind="ExternalInput").ap()
    # ffn weights
    io["wg"] = nc.dram_tensor("wg", [P, NF, ND, P], bf, kind="ExternalInput").ap()
    io["wu"] = nc.dram_tensor("wu", [P, NF, ND, P], bf, kind="ExternalInput").ap()
    io["wo"] = nc.dram_tensor("wo", [P, NF, D], bf, kind="ExternalInput").ap()
    # mask + constants
    io["mask"] = nc.dram_tensor("mask", [P, 2, 4, 256], bf,
                                kind="ExternalInput").ap()
    io["iden"] = nc.dram_tensor("iden", [P, P], bf, kind="ExternalInput").ap()
    io["consts"] = nc.dram_tensor("consts", [16, 9, P], bf,
                                  kind="ExternalInput").ap()
    io["outT"] = nc.dram_tensor("outT", [D, CHUNK], f32,
                                kind="ExternalOutput").ap()

    if os.environ.get("BASS_TAPS") == "1":
        for nm, shape in [("dbg_h1", [P, ND, LT]), ("dbg_q", [P, ND, CHUNK]),
                          ("dbg_k", [P, ND, LT]),
                          ("dbg_v", [P, (LT // P) * NH * (DH + 1)]),
                          ("dbg_att", [P, ND, CHUNK]),
                          ("dbg_gate", [P, ND, CHUNK]),
                          ("dbg_x2", [P, ND, CHUNK]),
                          ("dbg_h2", [P, ND, CHUNK])]:
            dt = f32 if nm == "dbg_x2" else bf
            io[nm] = nc.dram_tensor(nm, shape, dt, kind="ExternalOutput").ap()

    with tile.TileContext(nc) as tc:
        _emit(tc, io)

    nc.compile()
    return nc


def _emit(tc, io):
    from contextlib import ExitStack

    from concourse import mybir

    nc = tc.nc
    f32 = mybir.dt.float32
    bf = mybir.dt.bfloat16
    AF = mybir.ActivationFunctionType

    # round-robin DMA issue across the three queues; eng= overrides
    _q = [0]

    def dma(out, in_, eng=None):
        if eng is None:
            eng = (nc.sync, nc.scalar, nc.gpsimd)[_q[0] % 3]
            _q[0] += 1
        eng.dma_start(out=out, in_=in_)

    def tap(name, src_ap):
        if name in io:
            nc.sync.dma_start(out=io[name], in_=src_ap)

    FPRE = 4  # ffn fo-tiles prefetched during attention

    with ExitStack() as ctx:
        ctx.enter_context(nc.allow_low_precision(
            reason="bf16 matmul inputs; all accumulation stays fp32 in PSUM"))
        glob = ctx.enter_context(tc.tile_pool(name="glob", bufs=1))

        # ---- phase 0: global constants + input prefetch ------------------
        # xb goes first, split across SP/Act HWDGE; everything else mostly
        # on SP + Pool so Act's stream is free for rms1 compute early.
        consts = glob.tile([16, 9, P], bf, name="consts")
        dma(consts[:], io["consts"], eng=nc.sync)
        xTp = ctx.enter_context(tc.tile_pool(name="xTp", bufs=1))
        xT = xTp.tile([P, ND, LT], bf, name="xT")
        for i, a in enumerate(range(0, ND, 2)):
            dma(xT[:, a:a + 2, :], io["xb"][:, a:a + 2, :],
                eng=(nc.scalar if i % 2 else nc.sync))
        maskT = glob.tile([P, 2, 4, 256], bf, name="maskT")
        dma(maskT[:], io["mask"], eng=nc.gpsimd)

        onescol_f = glob.tile([P, 1], f32)
        nc.vector.memset(onescol_f, 1.0)
        onescol = glob.tile([P, 1], bf)
        nc.vector.tensor_copy(onescol[:], onescol_f[:])
        epsb = glob.tile([P, 1], f32)
        nc.vector.memset(epsb, EPS)
        shiftb = glob.tile([P, 1], f32)
        nc.vector.memset(shiftb, EXP_SHIFT)

        # prewarm the Sqrt Act table (rms1 critical path) while DMAs fly;
        # emitted after the phase-0 dma_starts so it doesn't delay Act's
        # own DMA issues
        warm = glob.tile([1, 1], f32, name="warm")
        nc.scalar.activation(warm[:], epsb[0:1, :], AF.Sqrt)

        # residual fp32 x (loads issued after the projection weights; only
        # needed at out-proj time)
        xF = glob.tile([P, ND, CHUNK], f32, name="xF")

        # identity stationary for PE-side residual accumulation into PSUM
        iden = glob.tile([P, P], bf, name="iden")
        dma(iden[:], io["iden"], eng=nc.sync)

        # projection weights: fully resident, DMAs issued right behind xb so
        # they win the per-queue FIFO race against later low-priority loads
        wqt = glob.tile([P, ND, ND, P], bf, name="wqt")
        wkt = glob.tile([P, ND, ND, P], bf, name="wkt")
        wgtt = glob.tile([P, ND, ND, P], bf, name="wgtt")
        wvt = glob.tile([P, 4, ND, 256], bf, name="wvt")
        for dst, src in ((wqt, "wq"), (wkt, "wk"), (wgtt, "wgt")):
            dma(dst[:, 0:4], io[src][:, 0:4], eng=nc.sync)
            dma(dst[:, 4:8], io[src][:, 4:8], eng=nc.gpsimd)
        dma(wvt[:, 0:2], io["wv"][:, 0:2], eng=nc.sync)
        dma(wvt[:, 2:4], io["wv"][:, 2:4], eng=nc.gpsimd)

        # ffn weights: persistent pools; wg/wu prefetch FPRE tiles early,
        # wo streams within the wo pass (bufs ahead) to avoid recycle stalls
        wgp = ctx.enter_context(tc.tile_pool(name="wgp", bufs=FPRE + 1))
        wup = ctx.enter_context(tc.tile_pool(name="wup", bufs=FPRE + 1))
        wop = ctx.enter_context(tc.tile_pool(name="wop", bufs=4))
        wg_tiles, wu_tiles, wo_tiles = {}, {}, {}

        def gu_fetch(fo):
            wg_tiles[fo] = wgp.tile([P, ND, P], bf, name="wgf")
            dma(wg_tiles[fo][:], io["wg"][:, fo])
            wu_tiles[fo] = wup.tile([P, ND, P], bf, name="wuf")
            dma(wu_tiles[fo][:], io["wu"][:, fo])

        def wo_fetch(fo):
            wo_tiles[fo] = wop.tile([P, D], bf, name="wof")
            dma(wo_tiles[fo][:], io["wo"][:, fo])

        # ---- phase 1: rmsnorm1 over all LT tokens ------------------------
        with ExitStack() as actx:
            h1p = actx.enter_context(tc.tile_pool(name="h1p", bufs=1))
            h1T = h1p.tile([P, ND, LT], bf, name="h1T")
            qT = actx.enter_context(tc.tile_pool(name="qTp", bufs=1)).tile(
                [P, ND, CHUNK], bf, name="qT")
            kT = actx.enter_context(tc.tile_pool(name="kTp", bufs=1)).tile(
                [P, ND, LT], bf, name="kT")
            vaug = actx.enter_context(tc.tile_pool(name="vaugp", bufs=1)).tile(
                [P, LT // P, NH, DH + 1], bf, name="vaug")
            gateT = actx.enter_context(tc.tile_pool(name="gateTp", bufs=1)).tile(
                [P, ND, CHUNK], bf, name="gateT")
            attT = actx.enter_context(tc.tile_pool(name="attTp", bufs=1)).tile(
                [P, ND, CHUNK], bf, name="attT")

            with ExitStack() as pctx:
                sqp = pctx.enter_context(tc.tile_pool(name="sqp", bufs=2))
                msp = pctx.enter_context(
                    tc.tile_pool(name="msp", bufs=2, space="PSUM"))
                rbp = pctx.enter_context(
                    tc.tile_pool(name="rbp", bufs=2, space="PSUM"))
                rowp = pctx.enter_context(tc.tile_pool(name="rowp", bufs=2))

                msg = [msp.tile([1, 384], f32, name=f"ms{g}", tag=f"ms{g}")
                       for g in range(2)]
                for a in range(ND):
                    sq = sqp.tile([P, LT], bf, name="sq")
                    nc.vector.tensor_mul(sq[:], xT[:, a, :], xT[:, a, :])
                    for g in range(2):
                        sl = slice(g * 384, (g + 1) * 384)
                        nc.tensor.matmul(msg[g][:], onescol[:], sq[:, sl],
                                         start=(a == 0), stop=(a == ND - 1))
                sr = rowp.tile([1, LT], f32, name="sr")
                for g in range(2):
                    nc.scalar.activation(sr[:, g * 384:(g + 1) * 384],
                                         msg[g][:], AF.Sqrt,
                                         bias=epsb[0:1, :], scale=1.0 / D)
                rro = rowp.tile([1, LT], bf, name="rro")
                nc.vector.reciprocal(rro[:], sr[:])
                rbc = rowp.tile([P, LT], bf, name="rbc")
                for g in range(2):
                    sl = slice(g * 384, (g + 1) * 384)
                    rb = rbp.tile([P, 384], f32, name="rb")
                    nc.tensor.matmul(rb[:], consts[0:1, 8, :], rro[:, sl],
                                     start=True, stop=True)
                    nc.vector.tensor_copy(rbc[:, sl], rb[:])
                for a in range(ND):
                    nc.vector.tensor_mul(h1T[:, a, :], xT[:, a, :], rbc[:])

            # v's ones-column init, late so it doesn't delay rms1 on DVE
            onescol_v = glob.tile([P, LT // P, NH, 1], f32)
            nc.vector.memset(onescol_v, 1.0)
            nc.vector.tensor_copy(vaug[:, :, :, DH:DH + 1], onescol_v[:])

            tap("dbg_h1", h1T[:])

            # ---- phase 2: q/k/v/gate projections (weights streamed) ------
            with ExitStack() as pctx:
                pj = pctx.enter_context(
                    tc.tile_pool(name="pj", bufs=4, space="PSUM"))

                # q^T [D, CHUNK] (own tokens only)
                for po in range(ND):
                    ps = pj.tile([P, CHUNK], f32, name="ps", tag="pj")
                    for pi in range(ND):
                        nc.tensor.matmul(ps[:], wqt[:, po, pi, :],
                                         h1T[:, pi, HALO:LT],
                                         start=(pi == 0), stop=(pi == ND - 1))
                    nc.scalar.copy(qT[:, po, :], ps[:])

                # k^T [D, LT]
                for po in range(ND):
                    for g in range(2):
                        sl = slice(g * 384, (g + 1) * 384)
                        ps = pj.tile([P, 384], f32, name="psk", tag="pj")
                        for pi in range(ND):
                            nc.tensor.matmul(ps[:], wkt[:, po, pi, :],
                                             h1T[:, pi, sl],
                                             start=(pi == 0), stop=(pi == ND - 1))
                        nc.scalar.copy(kT[:, po, sl], ps[:])

                # v [LT, D] (+ ones column), token-major
                for ng in range(4):
                    for tt in range(LT // P):
                        ps = pj.tile([P, 256], f32, name="psv", tag="pj")
                        for pi in range(ND):
                            nc.tensor.matmul(
                                ps[:], h1T[:, pi, tt * P:(tt + 1) * P],
                                wvt[:, ng, pi, :],
                                start=(pi == 0), stop=(pi == ND - 1))
                        nc.scalar.copy(
                            vaug[:, tt, 4 * ng:4 * (ng + 1), 0:DH],
                            ps[:].rearrange("p (h d) -> p h d", d=DH))

                # gate^T = sigmoid(h1 @ wgate.T)^T [D, CHUNK]
                for po in range(ND):
                    ps = pj.tile([P, CHUNK], f32, name="ps", tag="pj")
                    for pi in range(ND):
                        nc.tensor.matmul(ps[:], wgtt[:, po, pi, :],
                                         h1T[:, pi, HALO:LT],
                                         start=(pi == 0), stop=(pi == ND - 1))
                    nc.scalar.activation(gateT[:, po, :], ps[:], AF.Sigmoid)

            tap("dbg_q", qT[:])
            tap("dbg_k", kT[:])
            tap("dbg_v", vaug[:].rearrange("p a h d -> p (a h d)"))
            tap("dbg_gate", gateT[:])

            # ---- phase 3: windowed attention -----------------------------
            with ExitStack() as pctx:
                stp = pctx.enter_context(
                    tc.tile_pool(name="stp", bufs=2, space="PSUM"))
                avp = pctx.enter_context(
                    tc.tile_pool(name="avp", bufs=3, space="PSUM"))
                ptp = pctx.enter_context(tc.tile_pool(name="ptp", bufs=3))
                dnp = pctx.enter_context(tc.tile_pool(name="dnp", bufs=4))
                bcp = pctx.enter_context(tc.tile_pool(name="bcp", bufs=3))

                for qh in range(2):          # query halves of 256 tokens
                    kt0 = qh * 2             # first of 4 window key tiles
                    qs = slice(qh * 256, (qh + 1) * 256)

                    for h in range(NH):      # heads
                        poh, off = h // 2, (h % 2) * DH

                        st = stp.tile([P, 4, 256], f32, name="st")
                        for j in range(4):
                            nc.tensor.matmul(
                                st[:, j, :],
                                kT[off:off + DH, poh,
                                   (kt0 + j) * P:(kt0 + j + 1) * P],
                                qT[off:off + DH, poh, qs],
                                start=True, stop=True)

                        pt = ptp.tile([P, 4, 256], bf, name="pt")
                        nc.scalar.activation(pt[:], st[:], AF.Exp,
                                             bias=shiftb[:], scale=SCALE)
                        nc.vector.tensor_mul(pt[:], pt[:], maskT[:, qh, :, :])

                        av = avp.tile([DH + 1, 256], f32, name="av")
                        for j in range(4):
                            nc.tensor.matmul(
                                av[:], vaug[:, kt0 + j, h, :], pt[:, j, :],
                                start=(j == 0), stop=(j == 3))

                        # head output = av * (1/denom), fused from PSUM
                        srw = dnp.tile([1, 256], f32, name="srw")
                        nc.vector.reciprocal(srw[:], av[DH:DH + 1, :])
                        bc = bcp.tile([DH, 256], f32, name="bc")
                        nc.gpsimd.partition_broadcast(bc[:], srw[:],
                                                      channels=DH)
                        nc.vector.tensor_tensor(
                            attT[off:off + DH, poh, qs],
                            av[0:DH, :], bc[:], mybir.AluOpType.mult)

                        # stagger the low-priority loads behind attention:
                        # residual x during qh0, FFN prefetch during qh1
                        if qh == 0 and h < 4:
                            dma(xF[:, 2 * h:2 * h + 2, :],
                                io["xf"][:, 2 * h:2 * h + 2, :])
                        if qh == 1 and h < FPRE:
                            gu_fetch(h)

            tap("dbg_att", attT[:])

            # ---- phase 4: gate, out-proj, residual; rms2 reduce fused ----
            rro2 = glob.tile([1, CHUNK], bf, name="rro2")
            with ExitStack() as pctx:
                wsp4 = pctx.enter_context(tc.tile_pool(name="wsp4", bufs=4))
                sqp5 = pctx.enter_context(tc.tile_pool(name="sqp5", bufs=2))
                r2p = pctx.enter_context(tc.tile_pool(name="r2p", bufs=1))
                pj = pctx.enter_context(
                    tc.tile_pool(name="pj4", bufs=4, space="PSUM"))
                msp5 = pctx.enter_context(
                    tc.tile_pool(name="msp5", bufs=1, space="PSUM"))

                for po in range(ND):
                    nc.vector.tensor_mul(attT[:, po, :], attT[:, po, :],
                                         gateT[:, po, :])

                ms5 = msp5.tile([1, CHUNK], f32, name="ms5")
                for pjx in range(ND):
                    wt = wsp4.tile([P, ND, P], bf, name="wt4")
                    dma(wt[:], io["wo_a"][:, pjx])
                    ps = pj.tile([P, CHUNK], f32, name="ps4")
                    for po in range(ND):
                        nc.tensor.matmul(ps[:], wt[:, po, :],
                                         attT[:, po, :],
                                         start=(po == 0), stop=(po == ND - 1))
                    # x2 = x + attn_out, in place into xF; rms2 square+reduce
                    nc.vector.tensor_add(xF[:, pjx, :], ps[:], xF[:, pjx, :])
                    sq = sqp5.tile([P, CHUNK], f32, name="sq5")
                    nc.scalar.activation(sq[:], xF[:, pjx, :], AF.Square)
                    nc.tensor.matmul(ms5[:], onescol_f[:], sq[:],
                                     start=(pjx == 0), stop=(pjx == ND - 1))
                sr5 = r2p.tile([1, CHUNK], f32, name="sr5")
                nc.scalar.activation(sr5[:], ms5[:], AF.Sqrt,
                                     bias=epsb[0:1, :], scale=1.0 / D)
                nc.vector.reciprocal(rro2[:], sr5[:])

        # x2 = xF from here on.
        if "dbg_x2" in io:
            nc.sync.dma_start(out=io["dbg_x2"], in_=xF[:])

        # ---- phase 5: rmsnorm2 broadcast + swiglu ffn --------------------
        with ExitStack() as fctx:
            h2T = fctx.enter_context(tc.tile_pool(name="h2Tp", bufs=1)).tile(
                [P, ND, CHUNK], bf, name="h2T")
            prod = fctx.enter_context(tc.tile_pool(name="prodp", bufs=1)).tile(
                [P, NF, CHUNK], bf, name="prod")

            with ExitStack() as pctx:
                rbp = pctx.enter_context(
                    tc.tile_pool(name="rbp5", bufs=1, space="PSUM"))
                rowp = pctx.enter_context(tc.tile_pool(name="rowp5", bufs=1))

                rb = rbp.tile([P, CHUNK], f32, name="rb5")
                nc.tensor.matmul(rb[:], consts[0:1, 8, :], rro2[:],
                                 start=True, stop=True)
                rbc = rowp.tile([P, CHUNK], bf, name="rbc5")
                nc.vector.tensor_copy(rbc[:], rb[:])
                for a in range(ND):
                    nc.vector.tensor_mul(h2T[:, a, :], xF[:, a, :], rbc[:])

            tap("dbg_h2", h2T[:])

            # bf16 copy of x2 for the PE-side residual accumulation
            xFb = fctx.enter_context(tc.tile_pool(name="xFbp", bufs=1)).tile(
                [P, ND, CHUNK], bf, name="xFb")
            for a in range(ND):
                nc.vector.tensor_copy(xFb[:, a, :], xF[:, a, :])

            # gate/up products
            with ExitStack() as pctx:
                pg = pctx.enter_context(
                    tc.tile_pool(name="pg", bufs=2, space="PSUM"))
                pu = pctx.enter_context(
                    tc.tile_pool(name="pu", bufs=2, space="PSUM"))
                sgp = pctx.enter_context(tc.tile_pool(name="sgp", bufs=3))

                for fo in range(NF):
                    if fo >= FPRE:
                        gu_fetch(fo)
                    if fo >= NF - 4:
                        wo_fetch(fo - (NF - 4))
                    wgf, wuf = wg_tiles[fo], wu_tiles[fo]
                    gps = pg.tile([P, CHUNK], f32, name="gps")
                    for pi in range(ND):
                        nc.tensor.matmul(gps[:], wgf[:, pi, :], h2T[:, pi, :],
                                         start=(pi == 0), stop=(pi == ND - 1))
                    sg = sgp.tile([P, CHUNK], bf, name="sg")
                    nc.scalar.activation(sg[:], gps[:], AF.Silu)
                    ups = pu.tile([P, CHUNK], f32, name="ups")
                    for pi in range(ND):
                        nc.tensor.matmul(ups[:], wuf[:, pi, :], h2T[:, pi, :],
                                         start=(pi == 0), stop=(pi == ND - 1))
                    nc.vector.tensor_mul(prod[:, fo, :], sg[:], ups[:])

            # wo: out accumulation over all fo
            with ExitStack() as pctx:
                pz = pctx.enter_context(
                    tc.tile_pool(name="pz", bufs=1, space="PSUM"))
                outp = pctx.enter_context(tc.tile_pool(name="outp", bufs=1))

                z2 = [pz.tile([P, CHUNK], f32, name=f"z2_{j}", tag=f"z2_{j}")
                      for j in range(ND)]
                for fo in range(NF):
                    if fo + 4 < NF:
                        wo_fetch(fo + 4)
                    wof = wo_tiles[fo]
                    for j in range(ND):
                        nc.tensor.matmul(z2[j][:], wof[:, j * P:(j + 1) * P],
                                         prod[:, fo, :],
                                         start=(fo == 0), stop=False)
                # residual via identity matmul closes each accumulation
                ot = outp.tile([P, ND, CHUNK], f32, name="ot")
                for j in range(ND):
                    nc.tensor.matmul(z2[j][:], iden[:], xFb[:, j, :],
                                     start=False, stop=True)
                    nc.scalar.copy(ot[:, j, :], z2[j][:])
                    if j % 2 == 1:
                        dma(io["outT"].rearrange(
                            "(a p) t -> p a t", p=P)[:, j - 1:j + 1, :],
                            ot[:, j - 1:j + 1, :])


# --------------------------------------------------------------------------
# host-side sharding / unsharding
# --------------------------------------------------------------------------

def _bf16(x):
    import ml_dtypes
    return np.ascontiguousarray(x.astype(ml_dtypes.bfloat16))


def _tile_kmajor(w):
    """[D_in, D_out] -> [P, D_in//P (po-tiles of 128 out-cols), ...] layout
    [p, po, a, o] where w[a*P+p, po*P+o]."""
    din, dout = w.shape
    a, po = din // P, dout // P
    return np.ascontiguousarray(
        w.reshape(a, P, po, P).transpose(1, 2, 0, 3))


def _build_mask(chunk_start):
    """Band+validity mask in S^T layout: [c_within_tile, qhalf, ktile, r]."""
    m = np.zeros((2, 4, P, 256), np.float32)
    for qh in range(2):
        c = (np.arange(4 * P)[:, None])            # window key coord [0, 512)
        rr = np.arange(256)[None, :]
        band = (c >= rr + 1) & (c <= rr + WIN)
        valid = (chunk_start - 256 + qh * 256 + c) >= 0
        m[qh] = (band & valid).astype(np.float32).reshape(4, P, 256)
    return np.ascontiguousarray(m.transpose(2, 0, 1, 3))  # [P, 2, 4, 256]


def _build_consts():
    """[16, 9, 128]: [:, poh, :] one-hot head->partition maps; [0, 8, :] ones."""
    c = np.zeros((16, 9, P), np.float32)
    for poh in range(8):
        c[2 * poh, poh, 0:DH] = 1.0
        c[2 * poh + 1, poh, DH:2 * DH] = 1.0
    c[0, 8, :] = 1.0
    return c


def make_in_maps(x, ln1_w, qkv_w, gate_w, out_w, ln2_w, wg, wu, wo):
    tot = NH * DH
    # fold rmsnorm weights into the consuming projection weights
    wq_e = (qkv_w[0 * tot:1 * tot] * ln1_w[None, :]).T  # [D(in), D(out)]
    wk_e = (qkv_w[1 * tot:2 * tot] * ln1_w[None, :]).T
    wv_e = (qkv_w[2 * tot:3 * tot] * ln1_w[None, :]).T
    wgate_e = (gate_w * ln1_w[None, :]).T
    wout_e = out_w.T                                    # [tot, D]
    wg_e = (wg * ln2_w[None, :]).T                      # [D, DFF]
    wu_e = (wu * ln2_w[None, :]).T
    wo_e = wo.T                                         # [DFF, D]

    # pre-tiled device layouts
    wv_l = _tile_kmajor(wv_e).reshape(P, 4, 2, ND, P).transpose(
        0, 1, 3, 2, 4).reshape(P, 4, ND, 256)  # [p, ng, a, 256]
    wg_l = _tile_kmajor(wg_e)                            # [p, fo, a, o]
    wu_l = _tile_kmajor(wu_e)
    wo_l = np.ascontiguousarray(
        wo_e.reshape(NF, P, D).transpose(1, 0, 2))       # [p, fo, d]

    shared = {
        "wq": _bf16(_tile_kmajor(wq_e)),
        "wk": _bf16(_tile_kmajor(wk_e)),
        "wv": _bf16(wv_l),
        "wgt": _bf16(_tile_kmajor(wgate_e)),
        "wo_a": _bf16(_tile_kmajor(wout_e)),
        "wg": _bf16(wg_l),
        "wu": _bf16(wu_l),
        "wo": _bf16(wo_l),
        "consts": _bf16(_build_consts()),
        "iden": _bf16(np.eye(P, dtype=np.float32)),
    }

    in_maps = []
    for c in range(NCORES):
        b, ck = divmod(c, T // CHUNK)
        cs = ck * CHUNK
        xw = np.zeros((LT, D), np.float32)
        lo = cs - HALO
        xw[max(0, -lo):] = x[b, max(lo, 0):cs + CHUNK]
        m = dict(shared)
        xt = np.ascontiguousarray(xw.T)                  # [D, LT]
        m["xb"] = _bf16(xt.reshape(ND, P, LT).transpose(1, 0, 2))
        m["xf"] = np.ascontiguousarray(
            xt[:, HALO:].reshape(ND, P, CHUNK).transpose(1, 0, 2))
        m["mask"] = _bf16(_build_mask(cs))
        in_maps.append(m)
    return in_maps


def gather_output(results):
    out = np.empty((B, T, D), np.float32)
    for c in range(NCORES):
        b, ck = divmod(c, T // CHUNK)
        out[b, ck * CHUNK:(ck + 1) * CHUNK] = results[c]["outT"].T
    return out


def kernel(**inputs):
    from concourse.bass_utils import run_bass_kernel_spmd

    if "nc" not in _CACHE:
        _CACHE["nc"] = build_program()
    nc = _CACHE["nc"]

    in_maps = make_in_maps(**inputs)
    res = run_bass_kernel_spmd(nc, in_maps, core_ids=list(range(NCORES)))
    return gather_output(res.results)


if __name__ == "__main__":
    rng = np.random.default_rng(0)
    ins = {
        "x": rng.standard_normal((B, T, D), dtype=np.float32),
        "ln1_w": np.ones(D, np.float32),
        "qkv_w": rng.standard_normal((3 * NH * DH, D), dtype=np.float32) * 0.02,
        "gate_w": rng.standard_normal((NH * DH, D), dtype=np.float32) * 0.04,
        "out_w": rng.standard_normal((D, NH * DH), dtype=np.float32) * 0.04,
        "ln2_w": np.ones(D, np.float32),
        "wg": rng.standard_normal((DFF, D), dtype=np.float32) * 0.02,
        "wu": rng.standard_normal((DFF, D), dtype=np.float32) * 0.02,
        "wo": rng.standard_normal((D, DFF), dtype=np.float32) * 0.02,
    }
    out = kernel(**ins)
    print("out", out.shape, out.dtype, float(np.abs(out).mean()))


# revision 72
# speedup vs baseline: 13.4433x; 13.4433x over previous
"""Trainium2 Bass kernel for a sparse-attention (sliding-window) transformer block.

Reference computation (per batch b, token t):
    x = x + attn(rmsnorm(x, ln1_w));  attn = (windowed_softmax(qk)·v * sigmoid(gate)) @ out_w.T
    out = x + swiglu_ffn(rmsnorm(x, ln2_w))

Sharding: token-parallel across 8 cores (B=2 x 4 chunks of 512 tokens).  Each
core receives its 512 tokens plus the previous 256 tokens (sliding-window halo,
zeros for the first chunk) and recomputes K/V on the halo -> no collectives.

v2 versus the baseline:
  - all weights + most activations in bf16 (half the HBM traffic, 2x/4x DVE
    modes); the residual stream and PSUM accumulation stay fp32.
  - weights are pre-tiled on the host into the exact SBUF layouts so every
    DMA is partition-major contiguous (>=1KB descriptors).
  - DMA spread across all three issueable queues (SP / Act HWDGE + Pool
    SWDGE) round-robin, with FFN weights prefetched during attention.
  - softmax denominators are batched per query-half (one reciprocal + a
    one-hot PE matmul broadcast) instead of per-head gpsimd broadcasts.
  - elementwise work balanced across Act/DVE so the PE stream never starves.
"""

import os
import sys

import numpy as np

if "/opt/trn_rl_repo" not in sys.path:
    sys.path.insert(0, "/opt/trn_rl_repo")

# ---- problem constants (hardcoded; kernel.py must be self-contained) ----
D = 1024          # d_model
NH = 16           # heads
DH = 64           # head dim
DFF = 4096        # ffn hidden
WIN = 256         # sliding window
B, T = 2, 2048
EPS = 1e-6

NCORES = 8
CHUNK = 512       # own tokens per core
HALO = 256        # preceding-token halo
LT = CHUNK + HALO  # 768 local tokens (halo first)
P = 128
ND = D // P       # 8
NF = DFF // P     # 32

EXP_SHIFT = -20.0  # constant shift inside exp; softmax-invariant
SCALE = DH ** -0.5

_CACHE = {}


# --------------------------------------------------------------------------
# program builder
# --------------------------------------------------------------------------

def build_program():
    import concourse.bacc as bacc
    import concourse.tile as tile
    from concourse import mybir

    f32 = mybir.dt.float32
    bf = mybir.dt.bfloat16

    nc = bacc.Bacc("TRN2", target_bir_lowering=False, debug=False,
                   num_devices=NCORES)

    io = {}
    # activations
    io["xb"] = nc.dram_tensor("xb", [P, ND, LT], bf, kind="ExternalInput").ap()
    io["xf"] = nc.dram_tensor("xf", [P, ND, CHUNK], f32,
                              kind="ExternalInput").ap()
    # attention weights, pre-tiled [partition, tiles..., free]
    io["wq"] = nc.dram_tensor("wq", [P, ND, ND, P], bf, kind="ExternalInput").ap()
    io["wk"] = nc.dram_tensor("wk", [P, ND, ND, P], bf, kind="ExternalInput").ap()
    io["wv"] = nc.dram_tensor("wv", [P, 4, ND, 256], bf, kind="ExternalInput").ap()
    io["wgt"] = nc.dram_tensor("wgt", [P, ND, ND, P], bf, kind="ExternalInput").ap()
    io["wo_a"] = nc.dram_tensor("wo_a", [P, ND, ND, P], bf, kind="ExternalInput").ap()
    # ffn weights
    io["wg"] = nc.dram_tensor("wg", [P, NF, ND, P], bf, kind="ExternalInput").ap()
    io["wu"] = nc.dram_tensor("wu", [P, NF, ND, P], bf, kind="ExternalInput").ap()
    io["wo"] = nc.dram_tensor("wo", [P, NF, D], bf, kind="ExternalInput").ap()
    # mask + constants
    io["mask"] = nc.dram_tensor("mask", [P, 2, 4, 256], bf,
                                kind="ExternalInput").ap()
    io["iden"] = nc.dram_tensor("iden", [P, P], bf, kind="ExternalInput").ap()
    io["consts"] = nc.dram_tensor("consts", [16, 9, P], bf,
                                  kind="ExternalInput").ap()
    io["outT"] = nc.dram_tensor("outT", [D, CHUNK], f32,
                                kind="ExternalOutput").ap()

    if os.environ.get("BASS_TAPS") == "1":
        for nm, shape in [("dbg_h1", [P, ND, LT]), ("dbg_q", [P, ND, CHUNK]),
                          ("dbg_k", [P, ND, LT]),
                          ("dbg_v", [P, (LT // P) * NH * (DH + 1)]),
                          ("dbg_att", [P, ND, CHUNK]),
                          ("dbg_gate", [P, ND, CHUNK]),
                          ("dbg_x2", [P, ND, CHUNK]),
                          ("dbg_h2", [P, ND, CHUNK])]:
            dt = f32 if nm == "dbg_x2" else bf
            io[nm] = nc.dram_tensor(nm, shape, dt, kind="ExternalOutput").ap()

    with tile.TileContext(nc) as tc:
        _emit(tc, io)

    nc.compile()
    return nc


def _emit(tc, io):
    from contextlib import ExitStack

    from concourse import mybir

    nc = tc.nc
    f32 = mybir.dt.float32
    bf = mybir.dt.bfloat16
    AF = mybir.ActivationFunctionType

    # round-robin DMA issue across the three queues; eng= overrides
    _q = [0]

    def dma(out, in_, eng=None):
        if eng is None:
            eng = (nc.sync, nc.scalar, nc.gpsimd)[_q[0] % 3]
            _q[0] += 1
        eng.dma_start(out=out, in_=in_)

    def tap(name, src_ap):
        if name in io:
            nc.sync.dma_start(out=io[name], in_=src_ap)

    FPRE = 4  # ffn fo-tiles prefetched during attention

    with ExitStack() as ctx:
        ctx.enter_context(nc.allow_low_precision(
            reason="bf16 matmul inputs; all accumulation stays fp32 in PSUM"))
        glob = ctx.enter_context(tc.tile_pool(name="glob", bufs=1))

        # ---- phase 0: global constants + input prefetch ------------------
        # xb goes first, split across SP/Act HWDGE; everything else mostly
        # on SP + Pool so Act's stream is free for rms1 compute early.
        consts = glob.tile([16, 9, P], bf, name="consts")
        dma(consts[:], io["consts"], eng=nc.sync)
        xTp = ctx.enter_context(tc.tile_pool(name="xTp", bufs=1))
        xT = xTp.tile([P, ND, LT], bf, name="xT")
        for i, a in enumerate(range(0, ND, 2)):
            dma(xT[:, a:a + 2, :], io["xb"][:, a:a + 2, :],
                eng=(nc.scalar if i % 2 else nc.sync))
        maskT = glob.tile([P, 2, 4, 256], bf, name="maskT")
        dma(maskT[:], io["mask"], eng=nc.gpsimd)

        onescol_f = glob.tile([P, 1], f32)
        nc.vector.memset(onescol_f, 1.0)
        onescol = glob.tile([P, 1], bf)
        nc.vector.tensor_copy(onescol[:], onescol_f[:])
        epsb = glob.tile([P, 1], f32)
        nc.vector.memset(epsb, EPS)
        shiftb = glob.tile([P, 1], f32)
        nc.vector.memset(shiftb, EXP_SHIFT)

        # prewarm the Sqrt Act table (rms1 critical path) while DMAs fly;
        # emitted after the phase-0 dma_starts so it doesn't delay Act's
        # own DMA issues
        warm = glob.tile([1, 1], f32, name="warm")
        nc.scalar.activation(warm[:], epsb[0:1, :], AF.Sqrt)

        # residual fp32 x (loads issued after the projection weights; only
        # needed at out-proj time)
        xF = glob.tile([P, ND, CHUNK], f32, name="xF")

        # identity stationary for PE-side residual accumulation into PSUM
        iden = glob.tile([P, P], bf, name="iden")
        dma(iden[:], io["iden"], eng=nc.sync)

        # ffn weights: persistent pools; wg/wu prefetch FPRE tiles early,
        # wo streams within the wo pass (bufs ahead) to avoid recycle stalls
        wgp = ctx.enter_context(tc.tile_pool(name="wgp", bufs=FPRE + 1))
        wup = ctx.enter_context(tc.tile_pool(name="wup", bufs=FPRE + 1))
        wop = ctx.enter_context(tc.tile_pool(name="wop", bufs=4))
        wg_tiles, wu_tiles, wo_tiles = {}, {}, {}

        def gu_fetch(fo):
            wg_tiles[fo] = wgp.tile([P, ND, P], bf, name="wgf")
            dma(wg_tiles[fo][:], io["wg"][:, fo])
            wu_tiles[fo] = wup.tile([P, ND, P], bf, name="wuf")
            dma(wu_tiles[fo][:], io["wu"][:, fo])

        def wo_fetch(fo):
            wo_tiles[fo] = wop.tile([P, D], bf, name="wof")
            dma(wo_tiles[fo][:], io["wo"][:, fo])

        # ---- phase 1: rmsnorm1 over all LT tokens ------------------------
        with ExitStack() as actx:
            h1p = actx.enter_context(tc.tile_pool(name="h1p", bufs=1))
            h1T = h1p.tile([P, ND, LT], bf, name="h1T")
            qT = actx.enter_context(tc.tile_pool(name="qTp", bufs=1)).tile(
                [P, ND, CHUNK], bf, name="qT")
            kT = actx.enter_context(tc.tile_pool(name="kTp", bufs=1)).tile(
                [P, ND, LT], bf, name="kT")
            vaug = actx.enter_context(tc.tile_pool(name="vaugp", bufs=1)).tile(
                [P, LT // P, NH, DH + 1], bf, name="vaug")
            gateT = actx.enter_context(tc.tile_pool(name="gateTp", bufs=1)).tile(
                [P, ND, CHUNK], bf, name="gateT")
            attT = actx.enter_context(tc.tile_pool(name="attTp", bufs=1)).tile(
                [P, ND, CHUNK], bf, name="attT")

            # projection weights: resident for phases 1-2 only (innermost
            # scope, LIFO-freed before attention); DMAs issued right after
            # the phase-0 loads so they win the per-queue FIFO race
            wvt = actx.enter_context(tc.tile_pool(name="wvtp", bufs=1)).tile(
                [P, 4, ND, 256], bf, name="wvt")
            projw_sc = ExitStack()
            projw = projw_sc.enter_context(tc.tile_pool(name="projw", bufs=1))
            wqt = projw.tile([P, ND, ND, P], bf, name="wqt")
            wkt = projw.tile([P, ND, ND, P], bf, name="wkt")
            wgtt = projw.tile([P, ND, ND, P], bf, name="wgtt")
            for dst, src in ((wqt, "wq"), (wkt, "wk"), (wgtt, "wgt")):
                dma(dst[:, 0:4], io[src][:, 0:4], eng=nc.sync)
                dma(dst[:, 4:8], io[src][:, 4:8], eng=nc.gpsimd)
            dma(wvt[:, 0:2], io["wv"][:, 0:2], eng=nc.sync)
            dma(wvt[:, 2:4], io["wv"][:, 2:4], eng=nc.gpsimd)

            with ExitStack() as pctx:
                sqp = pctx.enter_context(tc.tile_pool(name="sqp", bufs=2))
                msp = pctx.enter_context(
                    tc.tile_pool(name="msp", bufs=2, space="PSUM"))
                rbp = pctx.enter_context(
                    tc.tile_pool(name="rbp", bufs=2, space="PSUM"))
                rowp = pctx.enter_context(tc.tile_pool(name="rowp", bufs=2))

                msg = [msp.tile([1, 384], f32, name=f"ms{g}", tag=f"ms{g}")
                       for g in range(2)]
                for a in range(ND):
                    sq = sqp.tile([P, LT], bf, name="sq")
                    nc.vector.tensor_mul(sq[:], xT[:, a, :], xT[:, a, :])
                    for g in range(2):
                        sl = slice(g * 384, (g + 1) * 384)
                        nc.tensor.matmul(msg[g][:], onescol[:], sq[:, sl],
                                         start=(a == 0), stop=(a == ND - 1))
                sr = rowp.tile([1, LT], f32, name="sr")
                for g in range(2):
                    nc.scalar.activation(sr[:, g * 384:(g + 1) * 384],
                                         msg[g][:], AF.Sqrt,
                                         bias=epsb[0:1, :], scale=1.0 / D)
                rro = rowp.tile([1, LT], bf, name="rro")
                nc.vector.reciprocal(rro[:], sr[:])
                rbc = rowp.tile([P, LT], bf, name="rbc")
                for g in range(2):
                    sl = slice(g * 384, (g + 1) * 384)
                    rb = rbp.tile([P, 384], f32, name="rb")
                    nc.tensor.matmul(rb[:], consts[0:1, 8, :], rro[:, sl],
                                     start=True, stop=True)
                    nc.vector.tensor_copy(rbc[:, sl], rb[:])
                for a in range(ND):
                    nc.vector.tensor_mul(h1T[:, a, :], xT[:, a, :], rbc[:])

            # v's ones-column init, late so it doesn't delay rms1 on DVE
            onescol_v = glob.tile([P, LT // P, NH, 1], f32)
            nc.vector.memset(onescol_v, 1.0)
            nc.vector.tensor_copy(vaug[:, :, :, DH:DH + 1], onescol_v[:])

            tap("dbg_h1", h1T[:])

            # ---- phase 2: q/k/v/gate projections (weights streamed) ------
            with ExitStack() as pctx:
                pj = pctx.enter_context(
                    tc.tile_pool(name="pj", bufs=4, space="PSUM"))

                # q^T [D, CHUNK] (own tokens only)
                for po in range(ND):
                    ps = pj.tile([P, CHUNK], f32, name="ps", tag="pj")
                    for pi in range(ND):
                        nc.tensor.matmul(ps[:], wqt[:, po, pi, :],
                                         h1T[:, pi, HALO:LT],
                                         start=(pi == 0), stop=(pi == ND - 1))
                    nc.scalar.copy(qT[:, po, :], ps[:])

                # k^T [D, LT]
                for po in range(ND):
                    for g in range(2):
                        sl = slice(g * 384, (g + 1) * 384)
                        ps = pj.tile([P, 384], f32, name="psk", tag="pj")
                        for pi in range(ND):
                            nc.tensor.matmul(ps[:], wkt[:, po, pi, :],
                                             h1T[:, pi, sl],
                                             start=(pi == 0), stop=(pi == ND - 1))
                        nc.scalar.copy(kT[:, po, sl], ps[:])

                # v [LT, D] (+ ones column), token-major; token-tiles 4-5
                # are only needed by qh1 attention and are deferred into the
                # qh0 head loop as PE filler
                for ng in range(4):
                    for tt in range(4):
                        ps = pj.tile([P, 256], f32, name="psv", tag="pj")
                        for pi in range(ND):
                            nc.tensor.matmul(
                                ps[:], h1T[:, pi, tt * P:(tt + 1) * P],
                                wvt[:, ng, pi, :],
                                start=(pi == 0), stop=(pi == ND - 1))
                        nc.scalar.copy(
                            vaug[:, tt, 4 * ng:4 * (ng + 1), 0:DH],
                            ps[:].rearrange("p (h d) -> p h d", d=DH))

                # gate^T = sigmoid(h1 @ wgate.T)^T [D, CHUNK]
                for po in range(ND):
                    ps = pj.tile([P, CHUNK], f32, name="ps", tag="pj")
                    for pi in range(ND):
                        nc.tensor.matmul(ps[:], wgtt[:, po, pi, :],
                                         h1T[:, pi, HALO:LT],
                                         start=(pi == 0), stop=(pi == ND - 1))
                    nc.scalar.activation(gateT[:, po, :], ps[:], AF.Sigmoid)

            tap("dbg_q", qT[:])
            tap("dbg_k", kT[:])
            tap("dbg_v", vaug[:].rearrange("p a h d -> p (a h d)"))
            tap("dbg_gate", gateT[:])

            projw_sc.close()  # free the projection weights' SBUF

            # ---- phase 3+4: attention with interleaved out-projection ----
            # token-half-0 out-proj chains run between qh1 attention heads
            # to fill PE gaps; gate-mul and softmax-divide are fused per head.
            rro2 = glob.tile([1, CHUNK], bf, name="rro2")
            with ExitStack() as pctx:
                pop = pctx.enter_context(
                    tc.tile_pool(name="pop", bufs=1, space="PSUM"))
                wsp4 = pctx.enter_context(tc.tile_pool(name="wsp4", bufs=8))
                sqp5 = pctx.enter_context(tc.tile_pool(name="sqp5", bufs=2))
                r2p = pctx.enter_context(tc.tile_pool(name="r2p", bufs=1))
                att_sc = ExitStack()
                stp = att_sc.enter_context(
                    tc.tile_pool(name="stp", bufs=2, space="PSUM"))
                avp = att_sc.enter_context(
                    tc.tile_pool(name="avp", bufs=2, space="PSUM"))
                ptp = att_sc.enter_context(tc.tile_pool(name="ptp", bufs=3))
                dnp = att_sc.enter_context(tc.tile_pool(name="dnp", bufs=2))
                bcp = att_sc.enter_context(tc.tile_pool(name="bcp", bufs=2))
                vdp = att_sc.enter_context(
                    tc.tile_pool(name="vdp", bufs=2, space="PSUM"))

                wo_at = [None] * ND

                def att_head(qh, h):
                    kt0 = qh * 2
                    qs = slice(qh * 256, (qh + 1) * 256)
                    poh, off = h // 2, (h % 2) * DH
                    st = stp.tile([P, 2, 256], f32, name="st")
                    st2 = stp.tile([P, 2, 256], f32, name="st")
                    for j in range(4):
                        nc.tensor.matmul(
                            (st if j < 2 else st2)[:, j % 2, :],
                            kT[off:off + DH, poh,
                               (kt0 + j) * P:(kt0 + j + 1) * P],
                            qT[off:off + DH, poh, qs],
                            start=True, stop=True)
                    pt = ptp.tile([P, 4, 256], bf, name="pt")
                    nc.scalar.activation(pt[:, 0:2, :], st[:], AF.Exp,
                                         bias=shiftb[:], scale=SCALE)
                    nc.scalar.activation(pt[:, 2:4, :], st2[:], AF.Exp,
                                         bias=shiftb[:], scale=SCALE)
                    nc.vector.tensor_mul(pt[:], pt[:], maskT[:, qh, :, :])
                    av = avp.tile([DH + 1, 256], f32, name="av")
                    for j in range(4):
                        nc.tensor.matmul(
                            av[:], vaug[:, kt0 + j, h, :], pt[:, j, :],
                            start=(j == 0), stop=(j == 3))
                    # head output = av * (1/denom) * gate, fused from PSUM
                    srw = dnp.tile([1, 256], f32, name="srw")
                    nc.vector.reciprocal(srw[:], av[DH:DH + 1, :])
                    bc = bcp.tile([DH, 256], f32, name="bc")
                    nc.gpsimd.partition_broadcast(bc[:], srw[:], channels=DH)
                    nc.vector.tensor_tensor(
                        attT[off:off + DH, poh, qs],
                        av[0:DH, :], bc[:], mybir.AluOpType.mult)
                    nc.vector.tensor_mul(
                        attT[off:off + DH, poh, qs],
                        attT[off:off + DH, poh, qs],
                        gateT[off:off + DH, poh, qs])

                po_pair = [None] * 2

                def oproj_chain(pjx, half):
                    qs = slice(half * 256, (half + 1) * 256)
                    if pjx % 2 == 0:
                        po_pair[(pjx // 2) % 2] = pop.tile(
                            [P, 2, 256], f32, name=f"po{(pjx // 2) % 2}",
                            tag=f"po{(pjx // 2) % 2}")
                    t = po_pair[(pjx // 2) % 2][:, pjx % 2, :]
                    for po in range(ND):
                        nc.tensor.matmul(t, wo_at[pjx][:, po, :],
                                         attT[:, po, qs],
                                         start=(po == 0), stop=(po == ND - 1))
                    # x2 = x + attn_out, in place into xF
                    nc.vector.tensor_add(xF[:, pjx, qs], t, xF[:, pjx, qs])

                for h in range(NH):
                    att_head(0, h)
                    if h < 4:
                        dma(xF[:, 2 * h:2 * h + 2, :],
                            io["xf"][:, 2 * h:2 * h + 2, :])
                        wo_at[h] = wsp4.tile([P, ND, P], bf, name="wt4")
                        dma(wo_at[h][:], io["wo_a"][:, h])
                    elif h - 4 < 4:
                        pjx = h - 4 + 4
                        wo_at[pjx] = wsp4.tile([P, ND, P], bf, name="wt4")
                        dma(wo_at[pjx][:], io["wo_a"][:, pjx])
                    if h >= 8:
                        # deferred v projection (token-tiles 4-5) as PE filler
                        dv = h - 8
                        ng, tt = dv // 2, 4 + dv % 2
                        ps = vdp.tile([P, 256], f32, name="psv2")
                        for pi in range(ND):
                            nc.tensor.matmul(
                                ps[:], h1T[:, pi, tt * P:(tt + 1) * P],
                                wvt[:, ng, pi, :],
                                start=(pi == 0), stop=(pi == ND - 1))
                        nc.scalar.copy(
                            vaug[:, tt, 4 * ng:4 * (ng + 1), 0:DH],
                            ps[:].rearrange("p (h d) -> p h d", d=DH))

                for h in range(NH):
                    att_head(1, h)
                    if h % 2 == 1:
                        oproj_chain(h // 2, 0)
                    if h < FPRE:
                        gu_fetch(h)

                att_sc.close()  # frees the attention PSUM banks
                msp5 = pctx.enter_context(
                    tc.tile_pool(name="msp5", bufs=1, space="PSUM"))
                ms5 = msp5.tile([1, CHUNK], f32, name="ms5")
                for pjx in range(ND):
                    oproj_chain(pjx, 1)
                    # rms2 square + reduce as each full x2 row-tile lands
                    sq = sqp5.tile([P, CHUNK], f32, name="sq5")
                    nc.scalar.activation(sq[:], xF[:, pjx, :], AF.Square)
                    nc.tensor.matmul(ms5[:], onescol_f[:], sq[:],
                                     start=(pjx == 0), stop=(pjx == ND - 1))

                sr5 = r2p.tile([1, CHUNK], f32, name="sr5")
                nc.scalar.activation(sr5[:], ms5[:], AF.Sqrt,
                                     bias=epsb[0:1, :], scale=1.0 / D)
                nc.vector.reciprocal(rro2[:], sr5[:])

            tap("dbg_att", attT[:])

        # x2 = xF from here on.
        if "dbg_x2" in io:
            nc.sync.dma_start(out=io["dbg_x2"], in_=xF[:])

        # ---- phase 5: rmsnorm2 broadcast + swiglu ffn --------------------
        with ExitStack() as fctx:
            h2T = fctx.enter_context(tc.tile_pool(name="h2Tp", bufs=1)).tile(
                [P, ND, CHUNK], bf, name="h2T")
            prod = fctx.enter_context(tc.tile_pool(name="prodp", bufs=1)).tile(
                [P, NF, CHUNK], bf, name="prod")

            with ExitStack() as pctx:
                rbp = pctx.enter_context(
                    tc.tile_pool(name="rbp5", bufs=1, space="PSUM"))
                rowp = pctx.enter_context(tc.tile_pool(name="rowp5", bufs=1))

                rb = rbp.tile([P, CHUNK], f32, name="rb5")
                nc.tensor.matmul(rb[:], consts[0:1, 8, :], rro2[:],
                                 start=True, stop=True)
                rbc = rowp.tile([P, CHUNK], bf, name="rbc5")
                nc.vector.tensor_copy(rbc[:], rb[:])
                for a in range(ND):
                    nc.vector.tensor_mul(h2T[:, a, :], xF[:, a, :], rbc[:])

            tap("dbg_h2", h2T[:])

            # bf16 copy of x2 for the PE-side residual accumulation
            xFb = fctx.enter_context(tc.tile_pool(name="xFbp", bufs=1)).tile(
                [P, ND, CHUNK], bf, name="xFb")
            for a in range(ND):
                nc.vector.tensor_copy(xFb[:, a, :], xF[:, a, :])

            # gate/up products
            with ExitStack() as pctx:
                pg = pctx.enter_context(
                    tc.tile_pool(name="pg", bufs=2, space="PSUM"))
                pu = pctx.enter_context(
                    tc.tile_pool(name="pu", bufs=2, space="PSUM"))
                sgp = pctx.enter_context(tc.tile_pool(name="sgp", bufs=3))

                for fo in range(NF):
                    if fo >= FPRE:
                        gu_fetch(fo)
                    if fo >= NF - 4:
                        wo_fetch(fo - (NF - 4))
                    wgf, wuf = wg_tiles[fo], wu_tiles[fo]
                    gps = pg.tile([P, CHUNK], f32, name="gps")
                    for pi in range(ND):
                        nc.tensor.matmul(gps[:], wgf[:, pi, :], h2T[:, pi, :],
                                         start=(pi == 0), stop=(pi == ND - 1))
                    sg = sgp.tile([P, CHUNK], bf, name="sg")
                    nc.scalar.activation(sg[:], gps[:], AF.Silu)
                    ups = pu.tile([P, CHUNK], f32, name="ups")
                    for pi in range(ND):
                        nc.tensor.matmul(ups[:], wuf[:, pi, :], h2T[:, pi, :],
                                         start=(pi == 0), stop=(pi == ND - 1))
                    nc.vector.tensor_mul(prod[:, fo, :], sg[:], ups[:])

            # wo: out accumulation over all fo
            with ExitStack() as pctx:
                pz = pctx.enter_context(
                    tc.tile_pool(name="pz", bufs=1, space="PSUM"))
                outp = pctx.enter_context(tc.tile_pool(name="outp", bufs=1))

                z2 = [pz.tile([P, CHUNK], f32, name=f"z2_{j}", tag=f"z2_{j}")
                      for j in range(ND)]
                for fo in range(NF):
                    if fo + 4 < NF:
                        wo_fetch(fo + 4)
                    wof = wo_tiles[fo]
                    for j in range(ND):
                        nc.tensor.matmul(z2[j][:], wof[:, j * P:(j + 1) * P],
                                         prod[:, fo, :],
                                         start=(fo == 0), stop=False)
                # residual via identity matmul closes each accumulation
                ot = outp.tile([P, ND, CHUNK], f32, name="ot")
                for j in range(ND):
                    nc.tensor.matmul(z2[j][:], iden[:], xFb[:, j, :],
                                     start=False, stop=True)
                    nc.scalar.copy(ot[:, j, :], z2[j][:])
                    if j % 2 == 1:
                        dma(io["outT"].rearrange(
                            "(a p) t -> p a t", p=P)[:, j - 1:j + 1, :],
                            ot[:, j - 1:j + 1, :])


# --------------------------------------------------------------------------
# host-side sharding / unsharding
# --------------------------------------------------------------------------

def _bf16(x):
    import ml_dtypes
    return np.ascontiguousarray(x.astype(ml_dtypes.bfloat16))


def _tile_kmajor(w):
    """[D_in, D_out] -> [P, D_in//P (po-tiles of 128 out-cols), ...] layout
    [p, po, a, o] where w[a*P+p, po*P+o]."""
    din, dout = w.shape
    a, po = din // P, dout // P
    return np.ascontiguousarray(
        w.reshape(a, P, po, P).transpose(1, 2, 0, 3))


def _build_mask(chunk_start):
    """Band+validity mask in S^T layout: [c_within_tile, qhalf, ktile, r]."""
    m = np.zeros((2, 4, P, 256), np.float32)
    for qh in range(2):
        c = (np.arange(4 * P)[:, None])            # window key coord [0, 512)
        rr = np.arange(256)[None, :]
        band = (c >= rr + 1) & (c <= rr + WIN)
        valid = (chunk_start - 256 + qh * 256 + c) >= 0
        m[qh] = (band & valid).astype(np.float32).reshape(4, P, 256)
    return np.ascontiguousarray(m.transpose(2, 0, 1, 3))  # [P, 2, 4, 256]


def _build_consts():
    """[16, 9, 128]: [:, poh, :] one-hot head->partition maps; [0, 8, :] ones."""
    c = np.zeros((16, 9, P), np.float32)
    for poh in range(8):
        c[2 * poh, poh, 0:DH] = 1.0
        c[2 * poh + 1, poh, DH:2 * DH] = 1.0
    c[0, 8, :] = 1.0
    return c


def make_in_maps(x, ln1_w, qkv_w, gate_w, out_w, ln2_w, wg, wu, wo):
    tot = NH * DH
    # fold rmsnorm weights into the consuming projection weights
    wq_e = (qkv_w[0 * tot:1 * tot] * ln1_w[None, :]).T  # [D(in), D(out)]
    wk_e = (qkv_w[1 * tot:2 * tot] * ln1_w[None, :]).T
    wv_e = (qkv_w[2 * tot:3 * tot] * ln1_w[None, :]).T
    wgate_e = (gate_w * ln1_w[None, :]).T
    wout_e = out_w.T                                    # [tot, D]
    wg_e = (wg * ln2_w[None, :]).T                      # [D, DFF]
    wu_e = (wu * ln2_w[None, :]).T
    wo_e = wo.T                                         # [DFF, D]

    # pre-tiled device layouts
    wv_l = _tile_kmajor(wv_e).reshape(P, 4, 2, ND, P).transpose(
        0, 1, 3, 2, 4).reshape(P, 4, ND, 256)  # [p, ng, a, 256]
    wg_l = _tile_kmajor(wg_e)                            # [p, fo, a, o]
    wu_l = _tile_kmajor(wu_e)
    wo_l = np.ascontiguousarray(
        wo_e.reshape(NF, P, D).transpose(1, 0, 2))       # [p, fo, d]

    shared = {
        "wq": _bf16(_tile_kmajor(wq_e)),
        "wk": _bf16(_tile_kmajor(wk_e)),
        "wv": _bf16(wv_l),
        "wgt": _bf16(_tile_kmajor(wgate_e)),
        "wo_a": _bf16(_tile_kmajor(wout_e)),
        "wg": _bf16(wg_l),
        "wu": _bf16(wu_l),
        "wo": _bf16(wo_l),
        "consts": _bf16(_build_consts()),
        "iden": _bf16(np.eye(P, dtype=np.float32)),
    }

    in_maps = []
    for c in range(NCORES):
        b, ck = divmod(c, T // CHUNK)
        cs = ck * CHUNK
        xw = np.zeros((LT, D), np.float32)
        lo = cs - HALO
        xw[max(0, -lo):] = x[b, max(lo, 0):cs + CHUNK]
        m = dict(shared)
        xt = np.ascontiguousarray(xw.T)                  # [D, LT]
        m["xb"] = _bf16(xt.reshape(ND, P, LT).transpose(1, 0, 2))
        m["xf"] = np.ascontiguousarray(
            xt[:, HALO:].reshape(ND, P, CHUNK).transpose(1, 0, 2))
        m["mask"] = _bf16(_build_mask(cs))
        in_maps.append(m)
    return in_maps


def gather_output(results):
    out = np.empty((B, T, D), np.float32)
    for c in range(NCORES):
        b, ck = divmod(c, T // CHUNK)
        out[b, ck * CHUNK:(ck + 1) * CHUNK] = results[c]["outT"].T
    return out


def kernel(**inputs):
    from concourse.bass_utils import run_bass_kernel_spmd

    if "nc" not in _CACHE:
        _CACHE["nc"] = build_program()
    nc = _CACHE["nc"]

    in_maps = make_in_maps(**inputs)
    res = run_bass_kernel_spmd(nc, in_maps, core_ids=list(range(NCORES)))
    return gather_output(res.results)


if __name__ == "__main__":
    rng = np.random.default_rng(0)
    ins = {
        "x": rng.standard_normal((B, T, D), dtype=np.float32),
        "ln1_w": np.ones(D, np.float32),
        "qkv_w": rng.standard_normal((3 * NH * DH, D), dtype=np.float32) * 0.02,
        "gate_w": rng.standard_normal((NH * DH, D), dtype=np.float32) * 0.04,
        "out_w": rng.standard_normal((D, NH * DH), dtype=np.float32) * 0.04,
        "ln2_w": np.ones(D, np.float32),
        "wg": rng.standard_normal((DFF, D), dtype=np.float32) * 0.02,
        "wu": rng.standard_normal((DFF, D), dtype=np.float32) * 0.02,
        "wo": rng.standard_normal((D, DFF), dtype=np.float32) * 0.02,
    }
    out = kernel(**ins)
    print("out", out.shape, out.dtype, float(np.abs(out).mean()))
